# revision 1
# baseline (speedup 1.0000x reference)
"""Trainium2 Bass kernel for ContentMultiheadAttention.

Reference computation (L=512, B=32, E=1024, H=16, hd=64):
  q,k,v = x @ W{q,k,v}.T + b    (torch F.linear convention)
  split heads -> [B*H, L, 64]; q /= 8
  S = q @ k.T;  S[mask] = -1e9;  P = softmax(S)
  O = P @ v -> merge heads -> out = O @ Wo.T + bo

Strategy: data-parallel over B across 8 cores (4 graphs/core). Per graph,
attention runs in S^T layout ([k, q]) so P^T feeds the PV matmul directly
(no transposes). Masking is multiplicative: P^T = exp(S^T) * M^T with a
binary bf16 keep-mask (exact zeros, no max-subtraction needed since
scores are bounded ~|4|). A ones-column appended to V yields softmax
denominators as an extra PV output row; normalization is a GPSIMD
partition-broadcast of 1/sums + one DVE multiply. Head pairs use PE row
groups 0/64 so their K=64 score matmuls can overlap in the array.
The emission is a cross-graph software pipeline: in-projection of graph
b+1 and out-projection of graph b-1 are woven between the attention head
pairs of graph b, keeping TensorE ~91% busy (ACT owns the exp stream).
All matmuls are bf16 with fp32 PSUM accumulation; softmax math is fp32.
"""

import numpy as np
import ml_dtypes

import concourse.mybir as mybir
import concourse.tile as tile
from concourse import bacc
from concourse import bass_utils

L, B, E, H = 512, 32, 1024, 16
HD = E // H  # 64
NCORES = 8
BPC = B // NCORES  # graphs per core

BF = mybir.dt.bfloat16
F32 = mybir.dt.float32
AF = mybir.ActivationFunctionType
ALU = mybir.AluOpType

_BUILT = {}


def _build_module():
    """Construct + compile the per-core Bacc program (same NEFF on all cores)."""
    nc = bacc.Bacc(None, target_bir_lowering=False, debug=False)

    # --- DRAM I/O (per core) ---
    # x*: [graph, p, ein_chunk, token] (X^T laid out for 128-partition tiles)
    xq = nc.dram_tensor("xq", [BPC, 128, 8, L], BF, kind="ExternalInput").ap()
    xk = nc.dram_tensor("xk", [BPC, 128, 8, L], BF, kind="ExternalInput").ap()
    xv = nc.dram_tensor("xv", [BPC, 128, 8, L], BF, kind="ExternalInput").ap()
    # mask^T as multiplicative binary (1=keep, 0=masked): [graph, p, kc, q]
    mneg = nc.dram_tensor("mneg", [BPC, 128, 4, L], BF, kind="ExternalInput").ap()
    # weights W*^T: [p, ein_chunk, e_out]
    wq = nc.dram_tensor("wq", [128, 8, E], BF, kind="ExternalInput").ap()
    wk = nc.dram_tensor("wk", [128, 8, E], BF, kind="ExternalInput").ap()
    wv = nc.dram_tensor("wv", [128, 8, E], BF, kind="ExternalInput").ap()
    wo = nc.dram_tensor("wo", [128, 8, E], BF, kind="ExternalInput").ap()
    # q/k biases per e_out partition: [p, eo_chunk]
    bq = nc.dram_tensor("bq", [128, 8], F32, kind="ExternalInput").ap()
    bk = nc.dram_tensor("bk", [128, 8], F32, kind="ExternalInput").ap()
    # effective output bias (bo + Wo @ bv): [1, E]
    effb = nc.dram_tensor("effb", [1, E], BF, kind="ExternalInput").ap()
    out = nc.dram_tensor("out", [BPC, L, E], F32, kind="ExternalOutput").ap()

    with tile.TileContext(nc) as tc:
        with (
            tc.tile_pool(name="wpool", bufs=1) as wpool,
            tc.tile_pool(name="xpool", bufs=1) as xpool,
            tc.tile_pool(name="gpool", bufs=2) as gpool,
            tc.tile_pool(name="spool", bufs=3) as spool,
            tc.tile_pool(name="ppsum", bufs=2, space="PSUM") as ppsum,
            tc.tile_pool(name="spsum", bufs=3, space="PSUM") as spsum,
            tc.tile_pool(name="opsum", bufs=3, space="PSUM") as opsum,
        ):
            # resident weights (wq first; the rest deferred behind graph-0
            # inputs so the first in-proj matmuls start ASAP)
            wq_sb = wpool.tile([128, 8, E], BF)
            wk_sb = wpool.tile([128, 8, E], BF)
            wv_sb = wpool.tile([128, 8, E], BF)
            wo_sb = wpool.tile([128, 8, E], BF)
            bq_sb = wpool.tile([128, 8], F32)
            bk_sb = wpool.tile([128, 8], F32)
            effb_sb = wpool.tile([1, E], BF)
            ones_sb = wpool.tile([1, 128], BF)
            nc.sync.dma_start(wq_sb[:], wq[:])
            nc.sync.dma_start(bq_sb[:], bq[:])

            def load_graph(b):
                st = {}
                st["xq"] = xpool.tile([128, 8, L], BF, tag="xq", name="xq")
                st["xk"] = xpool.tile([128, 8, L], BF, tag="xk", name="xk")
                st["xv"] = xpool.tile([128, 8, L], BF, tag="xv", name="xv")
                st["mneg"] = xpool.tile([128, 4, L], BF, tag="mneg", bufs=2, name="mneg")
                nc.sync.dma_start(st["xq"][:], xq[b])
                nc.sync.dma_start(st["xk"][:], xk[b])
                nc.sync.dma_start(st["xv"][:], xv[b])
                nc.sync.dma_start(st["mneg"][:], mneg[b])
                if b == 0:
                    nc.sync.dma_start(wk_sb[:], wk[:])
                    nc.sync.dma_start(bk_sb[:], bk[:])
                    nc.sync.dma_start(wv_sb[:], wv[:])
                    nc.sync.dma_start(wo_sb[:], wo[:])
                    nc.sync.dma_start(effb_sb[:], effb[:])
                    nc.vector.memset(ones_sb[:], 1.0)
                st["qt"] = gpool.tile([128, 8, L], BF, tag="qt", name="qt")
                st["kt"] = gpool.tile([128, 8, L], BF, tag="kt", name="kt")
                st["vx"] = gpool.tile([128, 4, H, HD + 1], BF, tag="vx", name="vx")
                st["oat"] = gpool.tile([128, 8, L], BF, tag="oat", name="oat")
                nc.vector.memset(st["vx"][:, :, :, HD], 1.0)
                return st

            def inproj_pieces(st):
                """24 emit-closures: 16 QT/KT psum groups + 8 V groups."""
                pieces = []
                for w_sb, xkey, dkey, bias_sb in (
                    (wq_sb, "xq", "qt", bq_sb),
                    (wk_sb, "xk", "kt", bk_sb),
                ):
                    for eo in range(8):
                        def qk_piece(w_sb=w_sb, xkey=xkey, dkey=dkey,
                                     bias_sb=bias_sb, eo=eo):
                            ps = ppsum.tile([128, 512], F32, tag="ppsum")
                            for ei in range(8):
                                nc.tensor.matmul(
                                    ps[:],
                                    w_sb[:, ei, eo * 128 : (eo + 1) * 128],
                                    st[xkey][:, ei, :],
                                    start=(ei == 0),
                                    stop=(ei == 7),
                                )
                            nc.scalar.activation(
                                st[dkey][:, eo, :], ps[:], AF.Identity,
                                bias=bias_sb[:, eo : eo + 1], scale=1.0,
                            )
                        pieces.append(qk_piece)
                for t4 in range(4):
                    for ec in range(2):
                        def v_piece(t4=t4, ec=ec):
                            ps = ppsum.tile([128, 512], F32, tag="ppsum")
                            for ei in range(8):
                                nc.tensor.matmul(
                                    ps[:],
                                    st["xv"][:, ei, t4 * 128 : (t4 + 1) * 128],
                                    wv_sb[:, ei, ec * 512 : (ec + 1) * 512],
                                    start=(ei == 0),
                                    stop=(ei == 7),
                                )
                            nc.scalar.activation(
                                st["vx"][:, t4, ec * 8 : (ec + 1) * 8, 0:HD],
                                ps.rearrange("p (h d) -> p h d", d=HD),
                                AF.Copy,
                            )
                        pieces.append(v_piece)
                return pieces

            def emit_scores(st, hp):
                pts = {0: [], 1: []}
                for kc in range(4):
                    sps_pair = []
                    for par in (0, 1):
                        po = par * 64
                        sps = spsum.tile([128, 512], F32, tag="spsum")
                        nc.tensor.matmul(
                            sps[:],
                            st["kt"][po : po + 64, hp, kc * 128 : (kc + 1) * 128],
                            st["qt"][po : po + 64, hp, :],
                            start=True,
                            stop=True,
                        )
                        sps_pair.append(sps)
                    for par in (0, 1):
                        pt = spool.tile([128, 512], BF, tag="pt", bufs=16)
                        nc.scalar.activation(pt[:], sps_pair[par][:], AF.Exp)
                        # zero masked entries (bf16 SBUF multiply)
                        nc.vector.tensor_tensor(
                            pt[:], pt[:], st["mneg"][:, kc, :], op=ALU.mult
                        )
                        pts[par].append(pt)
                return pts

            def emit_pv(st, hp, pts):
                for par in (0, 1):
                    h = 2 * hp + par
                    po = par * 64
                    ops = opsum.tile([65, 512], F32, tag="opsum")
                    for kc in range(4):
                        nc.tensor.matmul(
                            ops[:],
                            st["vx"][:, kc, h, :],
                            pts[par][kc][:],
                            start=(kc == 0),
                            stop=(kc == 3),
                        )
                    recip_t = spool.tile([65, 512], F32, tag="recip", bufs=2)
                    nc.vector.reciprocal(recip_t[64:65, :], ops[64:65, :])
                    nc.sync.dma_start(recip_t[0:1, :], recip_t[64:65, :])
                    rbc = spool.tile([64, 512], F32, tag="rbc", bufs=2)
                    nc.gpsimd.partition_broadcast(rbc[:], recip_t[0:1, :])
                    otmp = spool.tile([64, 512], BF, tag="otmp", bufs=2)
                    nc.vector.tensor_tensor(
                        otmp[:], ops[0:64, :], rbc[:], op=ALU.mult
                    )
                    nc.sync.dma_start(st["oat"][po : po + 64, hp, :], otmp[:])

            def emit_attention(st, pieces):
                """Head pairs, PV one pair behind scores, in-proj pieces of
                the NEXT graph woven between pairs to keep PE fed while ACT
                runs the exp stream."""
                prev_pts = None
                for hp in range(8):
                    pts = emit_scores(st, hp)
                    if prev_pts is not None:
                        emit_pv(st, hp - 1, prev_pts)
                    n_pop = (len(pieces) + (7 - hp)) // (8 - hp)
                    for _ in range(n_pop):
                        if pieces:
                            pieces.pop(0)()
                    prev_pts = pts
                emit_pv(st, 7, prev_pts)
                while pieces:
                    pieces.pop(0)()

            def outproj_pieces(st, b):
                pieces = []
                for t4 in range(4):
                    for ec in range(2):
                        def o_piece(t4=t4, ec=ec):
                            fps = ppsum.tile([128, 512], F32, tag="ppsum")
                            for eo in range(8):
                                nc.tensor.matmul(
                                    fps[:],
                                    st["oat"][:, eo, t4 * 128 : (t4 + 1) * 128],
                                    wo_sb[:, eo, ec * 512 : (ec + 1) * 512],
                                    start=(eo == 0),
                                    stop=False,
                                )
                            nc.tensor.matmul(
                                fps[:],
                                ones_sb[:],
                                effb_sb[:, ec * 512 : (ec + 1) * 512],
                                start=False,
                                stop=True,
                            )
                            f_sb = spool.tile([128, 512], F32, tag="fsb", bufs=2)
                            nc.scalar.activation(f_sb[:], fps[:], AF.Copy)
                            nc.sync.dma_start(
                                out[b, t4 * 128 : (t4 + 1) * 128,
                                    ec * 512 : (ec + 1) * 512],
                                f_sb[:],
                            )
                        pieces.append(o_piece)
                return pieces

            st = load_graph(0)
            for p in inproj_pieces(st):
                p()
            states = [st]
            for b in range(1, BPC):
                st_next = load_graph(b)
                pieces = inproj_pieces(st_next)
                if b >= 2:
                    pieces = pieces + outproj_pieces(states[b - 2], b - 2)
                emit_attention(states[b - 1], pieces)
                states.append(st_next)
            emit_attention(states[BPC - 1], outproj_pieces(states[BPC - 2], BPC - 2))
            for p in outproj_pieces(states[BPC - 1], BPC - 1):
                p()

    nc.compile()
    return nc


def _prep_inputs(query, key, value, attn_mask, in_proj_weight, in_proj_bias,
                 out_proj_weight, out_proj_bias):
    bf16 = ml_dtypes.bfloat16

    def xt_layout(x):  # [L, B, E] -> [B, 128, 8, L]
        return np.ascontiguousarray(
            x.reshape(L, B, 8, 128).transpose(1, 3, 2, 0)
        ).astype(bf16)

    def wt_layout(w):  # [e_out, e_in] -> W^T as [128, 8, e_out]
        return np.ascontiguousarray(
            w.T.reshape(8, 128, E).transpose(1, 0, 2)
        ).astype(bf16)

    Wq = in_proj_weight[0:E] / np.float32(np.sqrt(HD))
    Wk = in_proj_weight[E : 2 * E]
    Wv = in_proj_weight[2 * E : 3 * E]
    bq_e = in_proj_bias[0:E] / np.float32(np.sqrt(HD))
    bk_e = in_proj_bias[E : 2 * E]
    bv_e = in_proj_bias[2 * E : 3 * E]

    mneg = np.where(attn_mask, np.float32(0.0), np.float32(1.0))  # [B, q, k]
    # -> [B, k, q] -> [B, 128, 4, q]
    mneg = np.ascontiguousarray(
        mneg.transpose(0, 2, 1).reshape(B, 4, 128, L).transpose(0, 2, 1, 3)
    ).astype(bf16)

    effb = (out_proj_bias + out_proj_weight @ bv_e).astype(np.float32)

    host = {
        "xq": xt_layout(query),
        "xk": xt_layout(key),
        "xv": xt_layout(value),
        "mneg": mneg,
        "wq": wt_layout(Wq),
        "wk": wt_layout(Wk),
        "wv": wt_layout(Wv),
        "wo": wt_layout(out_proj_weight),
        "bq": np.ascontiguousarray(bq_e.reshape(8, 128).T).astype(np.float32),
        "bk": np.ascontiguousarray(bk_e.reshape(8, 128).T).astype(np.float32),
        "effb": np.ascontiguousarray(effb.reshape(1, E)).astype(bf16),
    }
    shared = {k: host[k] for k in ("wq", "wk", "wv", "wo", "bq", "bk", "effb")}
    in_maps = []
    for c in range(NCORES):
        sl = slice(c * BPC, (c + 1) * BPC)
        m = dict(shared)
        m["xq"] = np.ascontiguousarray(host["xq"][sl])
        m["xk"] = np.ascontiguousarray(host["xk"][sl])
        m["xv"] = np.ascontiguousarray(host["xv"][sl])
        m["mneg"] = np.ascontiguousarray(host["mneg"][sl])
        in_maps.append(m)
    return in_maps


def kernel(query, key, value, attn_mask, in_proj_weight, in_proj_bias,
           out_proj_weight, out_proj_bias, num_heads, _trace=False):
    query = np.asarray(query, dtype=np.float32)
    key = np.asarray(key, dtype=np.float32)
    value = np.asarray(value, dtype=np.float32)
    attn_mask = np.asarray(attn_mask)
    in_proj_weight = np.asarray(in_proj_weight, dtype=np.float32)
    in_proj_bias = np.asarray(in_proj_bias, dtype=np.float32)
    out_proj_weight = np.asarray(out_proj_weight, dtype=np.float32)
    out_proj_bias = np.asarray(out_proj_bias, dtype=np.float32)
    assert int(num_heads) == H

    if "nc" not in _BUILT:
        _BUILT["nc"] = _build_module()
    nc = _BUILT["nc"]

    in_maps = _prep_inputs(query, key, value, attn_mask, in_proj_weight,
                           in_proj_bias, out_proj_weight, out_proj_bias)
    res = bass_utils.run_bass_kernel_spmd(
        nc, in_maps, core_ids=list(range(NCORES)), trace=_trace
    )
    outs = np.stack([r["out"] for r in res.results])  # [8, BPC, L, E]
    # full[l, c*BPC+j, e] = outs[c, j, l, e]
    full = outs.transpose(2, 0, 1, 3).reshape(L, B, E)
    if _trace:
        return np.ascontiguousarray(full.astype(np.float32)), res
    return np.ascontiguousarray(full.astype(np.float32))

